# revision 3
# baseline (speedup 1.0000x reference)
"""DGN layer (gnn_message_passing) on 8 TRN2 NeuronCores.

Sharding: nodes split across 8 cores by destination range (graph parallel).
Host does index-only preprocessing (edge sort/bucketing, padding maps, dtype
casts, layout staging); every float op of the layer itself runs on device.

Bottleneck analysis (perfetto): the Q7 SWDGE descriptor generation inside
`dma_gather` costs ~7.5 ns per gathered 256B row and runs serially on the
Pool engine, so the kernel is gather-descriptor-bound.  This version
therefore uses ONE dma_gather per 128-dst block (the padded "mailbox"
layout, [128 dst, S_b slots, 128 feat]) and computes all three aggregations
from it:

  - sum_h:  PE accumulation of slot tiles via identity-lhsT matmuls into
    PSUM; padding slots replicate edge 0, corrected exactly afterwards with
    sum -= (S_b - deg) * mail[:,0,:].
  - dir:    per-slot diagonal-weight matmuls (lhsT = diag(w_s), built in one
    broadcasted DVE multiply); padding has w=0 via a staged 0/1 mask, so the
    weighted sum and its denominator are exact with no correction.
  - max:    pairwise DVE max tree over slots (padding replicates edge 0;
    deg==0 rows hit the zeros sentinel row, giving 0 as required).

Gathers use the SWDGE `dma_gather` ucode (int16 indices).  Since N=50k
exceeds the signed-int16 range, each core's blocks are grouped into Q>=4
contiguous segments; for each segment the host stages a renumbered bf16
sub-table of only the h rows that segment's edges reference (row 0 = zeros
sentinel), guaranteeing indices < 32768.

Epilogue per block: scale by 1/deg (resp 1/(den+1e-30)), PE transpose the
three [dst, feat] aggregates to [feat, dst], 3 matmuls against restacked W
(BN scale folded) -> y [128d, 384]; combine with amp/att per-node scalars,
snorm, BN shift, relu, residual.
"""

import math
import numpy as np

import ml_dtypes

import concourse.bass as bass
import concourse.bacc as bacc
import concourse.mybir as mybir
import concourse.tile as tile
from concourse.bass_utils import run_bass_kernel_spmd
from concourse.library_config import mlp

F32 = mybir.dt.float32
BF16 = mybir.dt.bfloat16
I16 = mybir.dt.int16
BF = ml_dtypes.bfloat16

AVG_D_LOG = float(np.log(33.0))
BN_EPS = 1e-5
D = 128
BLK = 128
TBL = 32768       # rows per segment sub-table (int16-addressable)


class _Cfg:
    def __init__(self, n, e, n_cores):
        self.N = n
        self.E = e
        self.NC = n_cores
        assert n % n_cores == 0
        self.NPC = n // n_cores
        self.NBLK = math.ceil(self.NPC / BLK)
        self.NPC_PAD = self.NBLK * BLK


def _wrap16(flat):
    """[NI] int array -> [128, NI//16] int16, 16-partition wrapped and
    replicated across the 8 Q7 groups (dma_gather index layout)."""
    ni = len(flat)
    assert ni % 16 == 0
    a = np.zeros((128, ni // 16), dtype=np.int16)
    i = np.arange(ni)
    a[i % 16, i // 16] = flat.astype(np.int16)
    for g in range(1, 8):
        a[g * 16:(g + 1) * 16] = a[0:16]
    return a


def _preprocess(cfg, h, eig, snorm_n, edge_src, edge_dst):
    """Index-only preprocessing + staging.  Returns (in_maps, meta)."""
    N, NC, NPC = cfg.N, cfg.NC, cfg.NPC
    NPC_PAD, NBLK = cfg.NPC_PAD, cfg.NBLK

    deg_all = np.bincount(edge_dst, minlength=N).astype(np.int64)
    eorder = np.argsort(edge_dst, kind="stable")
    esrc_s = edge_src[eorder].astype(np.int64)
    row_start = np.zeros(N + 1, dtype=np.int64)
    np.cumsum(deg_all, out=row_start[1:])

    eig0_bf = np.ascontiguousarray(eig[:, 0]).astype(BF)
    h_bf_full = h.astype(BF)

    # per-core degree-sorted node permutation (-1 = padding node)
    perms = []
    for c in range(NC):
        nodes = np.arange(c * NPC, (c + 1) * NPC, dtype=np.int64)
        p = nodes[np.argsort(-deg_all[nodes], kind="stable")]
        perm = np.full(NPC_PAD, -1, dtype=np.int64)
        perm[:NPC] = p
        perms.append(perm)
    perms = np.stack(perms)              # [NC, NPC_PAD]
    pdeg = np.where(perms >= 0, deg_all[np.clip(perms, 0, N - 1)], 0)

    # global (cross-core uniform) mailbox slots per block
    S_bs = [max(int(pdeg[:, b * BLK:(b + 1) * BLK].max()), 1)
            for b in range(NBLK)]
    SM_tot = sum(S_bs)
    moff = np.zeros(NBLK, dtype=np.int64)
    np.cumsum(S_bs[:-1], out=moff[1:])

    # segment (sub-table) assignment of blocks: Q contiguous groups;
    # grow Q until every (core, segment)'s distinct source count fits int16
    def seg_bounds(nseg):
        per = math.ceil(NBLK / nseg)
        return [(q * per, min((q + 1) * per, NBLK)) for q in range(nseg)]

    def srcs_of(c, b0, b1):
        rows = perms[c, b0 * BLK:b1 * BLK]
        rows = rows[rows >= 0]
        segs = [esrc_s[row_start[g]:row_start[g] + deg_all[g]] for g in rows]
        return np.unique(np.concatenate(segs)) if segs else np.array([], np.int64)

    Q = 4
    while True:
        ok = True
        uniqs = {}
        for c in range(NC):
            for q, (b0, b1) in enumerate(seg_bounds(Q)):
                u = srcs_of(c, b0, b1)
                if len(u) > TBL - 2:
                    ok = False
                    break
                uniqs[(c, q)] = u
            if not ok:
                break
        if ok:
            break
        Q += 1
        assert Q <= 16, "segmenting failed"
    bounds = seg_bounds(Q)
    seg_of_block = np.zeros(NBLK, dtype=np.int64)
    for q, (b0, b1) in enumerate(bounds):
        seg_of_block[b0:b1] = q

    in_maps = []
    for c in range(NC):
        perm = perms[c]
        dg = pdeg[c]

        # ---- segment tables + renumber maps ----
        tbls = np.zeros((Q, TBL, D), dtype=BF)
        remap = {}
        for q in range(Q):
            u = uniqs[(c, q)]
            tbls[q, 1:1 + len(u)] = h_bf_full[u]
            remap[q] = u            # sorted; renum = searchsorted+1

        def renum(q, srcs):
            return np.searchsorted(remap[q], srcs) + 1

        # ---- mailbox staging (wrapped int16, slot-major) + eig/mask ----
        idx_mail_w = np.zeros((128, SM_tot * 8), dtype=np.int16)
        a_mail = np.zeros((128, SM_tot), dtype=BF)
        mask_mail = np.zeros((128, SM_tot), dtype=BF)
        for b in range(NBLK):
            S_b, off = S_bs[b], moff[b]
            q = seg_of_block[b]
            flat = np.zeros((S_b, BLK), dtype=np.int64)   # [slot, dst]
            for d in range(BLK):
                r = b * BLK + d
                g = perm[r]
                k = dg[r]
                if g < 0 or k == 0:
                    continue
                raw = esrc_s[row_start[g]:row_start[g] + k]
                srcs = renum(q, raw)
                flat[:k, d] = srcs
                flat[k:, d] = srcs[0]
                a_mail[d, off:off + k] = eig0_bf[raw]
                mask_mail[d, off:off + k] = 1.0
            idx_mail_w[:, off * 8:(off + S_b) * 8] = _wrap16(flat.ravel())

        # ---- per-node scalars / residual ----
        safe = np.clip(perm, 0, N - 1)
        degf = dg.astype(np.float32)
        deg_t = np.ascontiguousarray(degf.reshape(NBLK, BLK).T)
        sn = np.where(perm >= 0, snorm_n[safe, 0], 0.0).astype(np.float32)
        snorm_t = np.ascontiguousarray(sn.reshape(NBLK, BLK).T)
        e0 = np.where(perm >= 0, eig0_bf[safe].astype(np.float32), 0.0)
        eig0d_t = np.ascontiguousarray(e0.reshape(NBLK, BLK).T).astype(np.float32)
        hin = np.where(perm[:, None] >= 0, h[safe], 0.0).astype(np.float32)

        m = dict(
            idx_mail=idx_mail_w, a_mail=a_mail, mask_mail=mask_mail,
            deg_t=deg_t, snorm_t=snorm_t, eig0d_t=eig0d_t, hin=hin,
        )
        for q in range(Q):
            m[f"tbl{q}"] = tbls[q]
        in_maps.append(m)

    meta = dict(perms=perms, S_bs=S_bs, moff=moff, SM_tot=SM_tot, Q=Q,
                seg_of_block=seg_of_block)
    return in_maps, meta


def _stage_consts(W, b, bn_gamma, bn_beta, bn_mean, bn_var):
    # W rows: c = i*384 + j*128 + f' (i = scale 0:id,1:amp,2:att;
    # j = agg 0:mean,1:max,2:dir).  wcat[:, j, i*128+f] = W[i*384+j*128+c, f]
    Wr = W.reshape(3, 3, 128, D)            # [i, j, c, f]
    wcat = np.ascontiguousarray(Wr.transpose(2, 1, 0, 3)).reshape(128, 3, 3 * D)
    bn = np.concatenate([bn_gamma, bn_beta, bn_mean, bn_var]).reshape(1, 4 * D)
    return dict(
        wcat=wcat.astype(np.float32),
        bvec=b.reshape(1, D).astype(np.float32),
        bn=bn.astype(np.float32),
        ident_bf=np.eye(128, dtype=BF),
    )


def _build_program(cfg, meta):
    NBLK, NPC_PAD = cfg.NBLK, cfg.NPC_PAD
    S_bs, moff = meta["S_bs"], meta["moff"]
    SM_tot, Q = meta["SM_tot"], meta["Q"]
    seg_of_block = meta["seg_of_block"]
    Smax = max(S_bs)
    AOT = mybir.AluOpType
    AFT = mybir.ActivationFunctionType

    nc = bacc.Bacc("TRN2", target_bir_lowering=False, debug=False)

    tbl_d = [nc.dram_tensor(f"tbl{q}", [TBL, D], BF16, kind="ExternalInput")
             for q in range(Q)]
    idx_mail = nc.dram_tensor("idx_mail", [128, SM_tot * 8], I16,
                              kind="ExternalInput")
    a_mail_d = nc.dram_tensor("a_mail", [128, SM_tot], BF16, kind="ExternalInput")
    mask_d = nc.dram_tensor("mask_mail", [128, SM_tot], BF16, kind="ExternalInput")
    deg_t = nc.dram_tensor("deg_t", [128, NBLK], F32, kind="ExternalInput")
    snorm_t = nc.dram_tensor("snorm_t", [128, NBLK], F32, kind="ExternalInput")
    eig0d_t = nc.dram_tensor("eig0d_t", [128, NBLK], F32, kind="ExternalInput")
    hin = nc.dram_tensor("hin", [NPC_PAD, D], F32, kind="ExternalInput")
    wcat = nc.dram_tensor("wcat", [128, 3, 3 * D], F32, kind="ExternalInput")
    bvec = nc.dram_tensor("bvec", [1, D], F32, kind="ExternalInput")
    bn = nc.dram_tensor("bn", [1, 4 * D], F32, kind="ExternalInput")
    ident_bf_d = nc.dram_tensor("ident_bf", [128, 128], BF16, kind="ExternalInput")

    out_d = nc.dram_tensor("out", [NPC_PAD, D], F32, kind="ExternalOutput")

    with tile.TileContext(nc) as tc:
        with (
            tc.tile_pool(name="stage", bufs=1) as stg,
            tc.tile_pool(name="const", bufs=1) as cst,
            tc.tile_pool(name="idxp", bufs=3) as idxp,
            tc.tile_pool(name="mail", bufs=2) as mailp,
            tc.tile_pool(name="diag", bufs=2) as diagp,
            tc.tile_pool(name="work", bufs=3) as wk,
            tc.tile_pool(name="ep", bufs=2) as ep,
            tc.tile_pool(name="pagg", bufs=2, space="PSUM") as pagg,
            tc.tile_pool(name="ptp", bufs=2, space="PSUM") as ptp,
            tc.tile_pool(name="py", bufs=2, space="PSUM") as py,
        ):
            nc.gpsimd.load_library(mlp)

            # ---------- staging loads ----------
            def load(dram, shape, dtype, pool=stg):
                t = pool.tile(shape, dtype, tag=dram.name)
                nc.sync.dma_start(t[:], dram[:])
                return t

            amail_s = load(a_mail_d, [128, SM_tot], BF16)
            mask_s = load(mask_d, [128, SM_tot], BF16)
            degt_s = load(deg_t, [128, NBLK], F32)
            snormt_s = load(snorm_t, [128, NBLK], F32)
            eig0dt_s = load(eig0d_t, [128, NBLK], F32)
            bvec_s = load(bvec, [1, D], F32)
            bn_s = load(bn, [1, 4 * D], F32)
            identbf_s = load(ident_bf_d, [128, 128], BF16, pool=cst)
            wcat_s = load(wcat, [128, 3, 3 * D], F32)

            # ---------- bn fold / constant prep (rows on partition 0) ----------
            g_r = bn_s[:, 0:D]
            beta_r = bn_s[:, D:2 * D]
            mean_r = bn_s[:, 2 * D:3 * D]
            var_r = bn_s[:, 3 * D:4 * D]
            bnsc = cst.tile([1, D], F32, tag="bnsc")
            eps_t = cst.tile([1, 1], F32, tag="eps_t")
            nc.gpsimd.memset(eps_t[:], BN_EPS)
            nc.scalar.activation(bnsc[:], var_r, AFT.Sqrt, bias=eps_t[:], scale=1.0)
            nc.vector.reciprocal(bnsc[:], bnsc[:])
            nc.vector.tensor_tensor(bnsc[:], bnsc[:], g_r, op=AOT.mult)
            shift = cst.tile([1, D], F32, tag="shift")       # beta - mean*scale
            nc.vector.tensor_tensor(shift[:], mean_r, bnsc[:], op=AOT.mult)
            nc.vector.tensor_tensor(shift[:], beta_r, shift[:], op=AOT.subtract)
            bprime = cst.tile([1, D], F32, tag="bprime")     # b * scale
            nc.vector.tensor_tensor(bprime[:], bvec_s[:], bnsc[:], op=AOT.mult)

            # broadcast const rows across partitions (DMA replicate via DRAM)
            rows_dram = nc.dram_tensor("cst_rows", [3, D], F32)
            nc.sync.dma_start(rows_dram[0:1, :], bnsc[:])
            nc.sync.dma_start(rows_dram[1:2, :], shift[:])
            nc.sync.dma_start(rows_dram[2:3, :], bprime[:])
            bnsc_bc = cst.tile([128, D], F32, tag="bnsc_bc")
            nc.sync.dma_start(bnsc_bc[:], rows_dram[0:1, :].to_broadcast([128, D]))
            shift_bc = cst.tile([128, D], F32, tag="shift_bc")
            nc.sync.dma_start(shift_bc[:], rows_dram[1:2, :].to_broadcast([128, D]))
            bprime_bc = cst.tile([128, D], F32, tag="bprime_bc")
            nc.sync.dma_start(bprime_bc[:], rows_dram[2:3, :].to_broadcast([128, D]))

            # wcat_bf = wcat * bn_scale -> bf16
            wcat_bf = cst.tile([128, 3, 3 * D], BF16, tag="wcatbf")
            nc.vector.tensor_tensor(
                wcat_bf[:].rearrange("p j (i d) -> p j i d", i=3),
                wcat_s[:].rearrange("p j (i d) -> p j i d", i=3),
                bnsc_bc[:, None, None, :].to_broadcast([128, 3, 3, D]),
                op=AOT.mult)

            for b in range(NBLK):
                S_b, mo = S_bs[b], int(moff[b])
                tdram = tbl_d[int(seg_of_block[b])]

                # ======== mailbox gather ========
                im = idxp.tile([128, Smax * 8], I16, tag="im")
                nc.sync.dma_start(im[:, 0:S_b * 8],
                                  idx_mail[:, mo * 8:(mo + S_b) * 8])
                mail = mailp.tile([128, Smax, D], BF16, tag="mail")
                nc.gpsimd.dma_gather(mail[:, 0:S_b, :], tdram[:],
                                     im[:, 0:S_b * 8], S_b * 128, S_b * 128,
                                     D, single_packet=False)

                # ======== edge weights w = mask * |a_src - a_dst| ========
                wv = wk.tile([128, Smax], BF16, tag="wv")
                nc.vector.tensor_scalar(wv[:, 0:S_b], amail_s[:, mo:mo + S_b],
                                        eig0dt_s[:, b:b + 1], None,
                                        op0=AOT.subtract)
                nc.scalar.activation(wv[:, 0:S_b], wv[:, 0:S_b], AFT.Abs)
                nc.vector.tensor_tensor(wv[:, 0:S_b], wv[:, 0:S_b],
                                        mask_s[:, mo:mo + S_b], op=AOT.mult)
                den = wk.tile([128, 1], F32, tag="den")
                nc.vector.tensor_reduce(den[:], wv[:, 0:S_b],
                                        axis=mybir.AxisListType.X, op=AOT.add)
                nc.vector.tensor_scalar(den[:], den[:], 1e-30, None, op0=AOT.add)
                rden = wk.tile([128, 1], F32, tag="rden")
                nc.vector.reciprocal(rden[:], den[:])

                # diag_all[:, s, :] = diag weights for slot s (identity * w_s)
                diag = diagp.tile([128, Smax, 128], BF16, tag="diag")
                nc.vector.tensor_tensor(
                    diag[:, 0:S_b, :],
                    identbf_s[:, None, :].to_broadcast([128, S_b, 128]),
                    wv[:, 0:S_b, None].to_broadcast([128, S_b, 128]),
                    op=AOT.mult)

                # ======== PE slot accumulation: sum and dir ========
                ps_sum = pagg.tile([128, D], F32, tag="ps_sum")
                ps_dir = pagg.tile([128, D], F32, tag="ps_dir")
                for s in range(S_b):
                    nc.tensor.matmul(ps_sum[:], identbf_s[:], mail[:, s, :],
                                     start=(s == 0), stop=(s == S_b - 1),
                                     skip_group_check=True)
                for s in range(S_b):
                    nc.tensor.matmul(ps_dir[:], diag[:, s, :], mail[:, s, :],
                                     start=(s == 0), stop=(s == S_b - 1),
                                     skip_group_check=True)

                # ======== per-node scalar columns ========
                deg_col = degt_s[:, b:b + 1]
                degsafe = wk.tile([128, 1], F32, tag="degsafe")
                nc.vector.tensor_scalar(degsafe[:], deg_col, 1.0, None,
                                        op0=AOT.max)
                nrn = wk.tile([128, 1], F32, tag="nrn")      # -1/deg_safe
                nc.vector.reciprocal(nrn[:], degsafe[:])
                nc.vector.tensor_scalar(nrn[:], nrn[:], -1.0, None, op0=AOT.mult)
                cnt_col = wk.tile([128, 1], F32, tag="cnt")  # S_b - deg
                nc.vector.tensor_scalar(cnt_col[:], deg_col, -1.0, float(S_b),
                                        op0=AOT.mult, op1=AOT.add)
                logd_col = wk.tile([128, 1], F32, tag="logd_col")
                nc.scalar.activation(logd_col[:], deg_col, AFT.Ln,
                                     bias=1.0, scale=1.0)
                amp_col = wk.tile([128, 1], F32, tag="amp_col")
                nc.vector.tensor_scalar(amp_col[:], logd_col[:],
                                        1.0 / AVG_D_LOG, None, op0=AOT.mult)
                att_col = wk.tile([128, 1], F32, tag="att_col")
                nc.vector.tensor_scalar(att_col[:], logd_col[:], 1e-6, None,
                                        op0=AOT.max)
                nc.vector.reciprocal(att_col[:], att_col[:])
                nc.vector.tensor_scalar(att_col[:], att_col[:], AVG_D_LOG, None,
                                        op0=AOT.mult)

                # ======== padding correction + scaling -> bf16 [d, f] ========
                # mean = -(cnt*mail0 - sum) / deg_safe
                tmp = ep.tile([128, D], F32, tag="tmp")
                nc.vector.scalar_tensor_tensor(
                    tmp[:], mail[:, 0, :], cnt_col[:], ps_sum[:],
                    op0=AOT.mult, op1=AOT.subtract)
                mean_bf = ep.tile([128, D], BF16, tag="mean_bf")
                nc.vector.tensor_scalar(mean_bf[:], tmp[:], nrn[:], None,
                                        op0=AOT.mult)
                dir_bf = ep.tile([128, D], BF16, tag="dir_bf")
                nc.vector.tensor_scalar(dir_bf[:], ps_dir[:], rden[:], None,
                                        op0=AOT.mult)

                # ======== max tree (in place; after mail0 was consumed) ========
                n = S_b
                while n > 1:
                    h1 = (n + 1) // 2
                    nc.vector.tensor_tensor(
                        mail[:, 0:h1, :], mail[:, 0:h1, :],
                        mail[:, n - h1:n, :], op=AOT.max)
                    n = h1

                # ======== transpose [d,f] -> [f,d] ========
                rawT = []
                for j, src in enumerate((mean_bf[:], mail[:, 0, :], dir_bf[:])):
                    tp = ptp.tile([128, 128], BF16, tag="tp")
                    nc.tensor.transpose(tp[:], src, identbf_s[:])
                    rt = ep.tile([128, 128], BF16, tag=f"rawT{j}")
                    nc.scalar.copy(rt[:], tp[:])
                    rawT.append(rt)

                # ======== final matmuls + combine ========
                y_ps = py.tile([128, 3 * D], F32, tag="y")
                for j, rt in enumerate(rawT):
                    nc.tensor.matmul(y_ps[:], rt[:], wcat_bf[:, j, :],
                                     start=(j == 0), stop=(j == 2))

                y1_sb = ep.tile([128, D], F32, tag="y1_sb")
                nc.scalar.copy(y1_sb[:], y_ps[:, 0:D])
                u = ep.tile([128, D], F32, tag="u")
                nc.vector.scalar_tensor_tensor(
                    u[:], y_ps[:, D:2 * D], amp_col[:], y1_sb[:],
                    op0=AOT.mult, op1=AOT.add)
                v = ep.tile([128, D], F32, tag="v")
                nc.vector.scalar_tensor_tensor(
                    v[:], y_ps[:, 2 * D:3 * D], att_col[:], u[:],
                    op0=AOT.mult, op1=AOT.add)
                nc.vector.tensor_tensor(v[:], v[:], bprime_bc[:], op=AOT.add)
                nc.vector.scalar_tensor_tensor(
                    v[:], v[:], snormt_s[:, b:b + 1], shift_bc[:],
                    op0=AOT.mult, op1=AOT.add)
                hin_t = ep.tile([128, D], F32, tag="hin")
                nc.sync.dma_start(hin_t[:], hin[b * BLK:(b + 1) * BLK, :])
                out_t = ep.tile([128, D], F32, tag="out")
                nc.vector.scalar_tensor_tensor(
                    out_t[:], v[:], 0.0, hin_t[:], op0=AOT.max, op1=AOT.add)
                nc.sync.dma_start(out_d[b * BLK:(b + 1) * BLK, :], out_t[:])

    nc.compile()
    return nc


_CACHE = {}


def _run(h, eig, snorm_n, W, b, bn_gamma, bn_beta, bn_mean, bn_var,
         edge_src, edge_dst, n_cores=8, trace=False, sim=False):
    N, E = h.shape[0], edge_src.shape[0]
    cfg = _Cfg(N, E, n_cores)
    in_maps, meta = _preprocess(cfg, h, eig, snorm_n, edge_src, edge_dst)
    consts = _stage_consts(W, b, bn_gamma, bn_beta, bn_mean, bn_var)
    for m in in_maps:
        m.update(consts)

    key = (N, E, n_cores, meta["Q"], tuple(meta["S_bs"]))
    if key not in _CACHE:
        _CACHE[key] = _build_program(cfg, meta)
    nc = _CACHE[key]

    if sim:
        from concourse.bass_interp import CoreSim
        csim = CoreSim(nc)
        for k, v in in_maps[0].items():
            csim.tensor(k)[:] = v
        csim.simulate()
        results = [{"out": np.array(csim.tensor("out"))}]
        n_out = 1
        res = None
    else:
        res = run_bass_kernel_spmd(nc, in_maps, core_ids=list(range(n_cores)),
                                   trace=trace)
        results = res.results
        n_out = n_cores

    out = np.empty((N, D), dtype=np.float32)
    for c in range(n_out):
        perm = meta["perms"][c]
        oc = results[c]["out"]
        valid = perm >= 0
        out[perm[valid]] = oc[valid]
    return out, res


def kernel(**inputs):
    out, _ = _run(
        np.asarray(inputs["h"]), np.asarray(inputs["eig"]),
        np.asarray(inputs["snorm_n"]), np.asarray(inputs["W"]),
        np.asarray(inputs["b"]), np.asarray(inputs["bn_gamma"]),
        np.asarray(inputs["bn_beta"]), np.asarray(inputs["bn_mean"]),
        np.asarray(inputs["bn_var"]), np.asarray(inputs["edge_src"]),
        np.asarray(inputs["edge_dst"]))
    return out


# revision 10
# speedup vs baseline: 3.9132x; 3.9132x over previous
"""DGN layer (gnn_message_passing) on 8 TRN2 NeuronCores.

Sharding: nodes split across 8 cores by destination range (graph parallel).
Host does index-only preprocessing (edge sort/bucketing, padding maps, dtype
casts, layout staging); every float op of the layer itself runs on device.

Bottleneck analysis (perfetto): the Q7 SWDGE descriptor generation inside
`dma_gather` costs ~7.5 ns per gathered 256B row and runs serially on the
Pool engine, so the kernel is gather-descriptor-bound.  This version
therefore uses ONE dma_gather per 128-dst block (the padded "mailbox"
layout, [128 dst, S_b slots, 128 feat]) and computes all three aggregations
from it:

  - sum_h:  PE accumulation of slot tiles via identity-lhsT matmuls into
    PSUM; padding slots replicate edge 0, corrected exactly afterwards with
    sum -= (S_b - deg) * mail[:,0,:].
  - dir:    per-slot diagonal-weight matmuls (lhsT = diag(w_s), built in one
    broadcasted DVE multiply); padding has w=0 via a staged 0/1 mask, so the
    weighted sum and its denominator are exact with no correction.
  - max:    pairwise DVE max tree over slots (padding replicates edge 0;
    deg==0 rows hit the zeros sentinel row, giving 0 as required).

Gathers use the SWDGE `dma_gather` ucode (int16 indices).  Since N=50k
exceeds the signed-int16 range, each core's blocks are grouped into Q>=4
contiguous segments; for each segment the host stages a renumbered bf16
sub-table of only the h rows that segment's edges reference (row 0 = zeros
sentinel), guaranteeing indices < 32768.

Epilogue per block: scale by 1/deg (resp 1/(den+1e-30)), PE transpose the
three [dst, feat] aggregates to [feat, dst], 3 matmuls against restacked W
(BN scale folded) -> y [128d, 384]; combine with amp/att per-node scalars,
snorm, BN shift, relu, residual.
"""

import math
import numpy as np

import ml_dtypes

import concourse.bass as bass
import concourse.bacc as bacc
import concourse.mybir as mybir
import concourse.tile as tile
from concourse.bass_utils import run_bass_kernel_spmd
from concourse.library_config import mlp

F32 = mybir.dt.float32
BF16 = mybir.dt.bfloat16
I16 = mybir.dt.int16
BF = ml_dtypes.bfloat16

AVG_D_LOG = float(np.log(33.0))
BN_EPS = 1e-5
D = 128
BLK = 128
TBL = 32768       # rows per segment sub-table (int16-addressable)


class _Cfg:
    def __init__(self, n, e, n_cores):
        self.N = n
        self.E = e
        self.NC = n_cores
        assert n % n_cores == 0
        self.NPC = n // n_cores
        self.NBLK = math.ceil(self.NPC / BLK)
        self.NPC_PAD = self.NBLK * BLK


def _wrap16(flat):
    """[NI] int array -> [128, NI//16] int16, 16-partition wrapped and
    replicated across the 8 Q7 groups (dma_gather index layout)."""
    ni = len(flat)
    assert ni % 16 == 0
    a = np.zeros((128, ni // 16), dtype=np.int16)
    i = np.arange(ni)
    a[i % 16, i // 16] = flat.astype(np.int16)
    for g in range(1, 8):
        a[g * 16:(g + 1) * 16] = a[0:16]
    return a


def _preprocess(cfg, h, eig, snorm_n, edge_src, edge_dst):
    """Index-only preprocessing + staging.  Returns (in_maps, meta)."""
    N, NC, NPC = cfg.N, cfg.NC, cfg.NPC
    NPC_PAD, NBLK = cfg.NPC_PAD, cfg.NBLK

    deg_all = np.bincount(edge_dst, minlength=N).astype(np.int64)
    eorder = np.argsort(edge_dst, kind="stable")
    esrc_s = edge_src[eorder].astype(np.int64)
    row_start = np.zeros(N + 1, dtype=np.int64)
    np.cumsum(deg_all, out=row_start[1:])

    eig0_bf = np.ascontiguousarray(eig[:, 0]).astype(BF)
    h_bf_full = h.astype(BF)

    # per-core degree-sorted node permutation (-1 = padding node)
    perms = []
    for c in range(NC):
        nodes = np.arange(c * NPC, (c + 1) * NPC, dtype=np.int64)
        p = nodes[np.argsort(-deg_all[nodes], kind="stable")]
        perm = np.full(NPC_PAD, -1, dtype=np.int64)
        perm[:NPC] = p
        perms.append(perm)
    perms = np.stack(perms)              # [NC, NPC_PAD]
    pdeg = np.where(perms >= 0, deg_all[np.clip(perms, 0, N - 1)], 0)

    # global (cross-core uniform) mailbox slots per block; block offsets are
    # aligned to 32 slots so every staged slice starts 64B-aligned (DVE fast
    # path)
    S_bs = [max(int(pdeg[:, b * BLK:(b + 1) * BLK].max()), 1)
            for b in range(NBLK)]
    moff = np.zeros(NBLK, dtype=np.int64)
    for b in range(1, NBLK):
        moff[b] = (moff[b - 1] + S_bs[b - 1] + 31) // 32 * 32
    SM_tot = int(moff[-1] + S_bs[-1])

    # segment (sub-table) assignment of blocks: Q contiguous groups;
    # grow Q until every (core, segment)'s distinct source count fits int16
    def seg_bounds(nseg):
        per = math.ceil(NBLK / nseg)
        return [(q * per, min((q + 1) * per, NBLK)) for q in range(nseg)]

    def srcs_of(c, b0, b1):
        rows = perms[c, b0 * BLK:b1 * BLK]
        rows = rows[rows >= 0]
        segs = [esrc_s[row_start[g]:row_start[g] + deg_all[g]] for g in rows]
        return np.unique(np.concatenate(segs)) if segs else np.array([], np.int64)

    Q = 4
    while True:
        ok = True
        uniqs = {}
        for c in range(NC):
            for q, (b0, b1) in enumerate(seg_bounds(Q)):
                u = srcs_of(c, b0, b1)
                if len(u) > TBL - 2:
                    ok = False
                    break
                uniqs[(c, q)] = u
            if not ok:
                break
        if ok:
            break
        Q += 1
        assert Q <= 16, "segmenting failed"
    bounds = seg_bounds(Q)
    seg_of_block = np.zeros(NBLK, dtype=np.int64)
    for q, (b0, b1) in enumerate(bounds):
        seg_of_block[b0:b1] = q

    in_maps = []
    for c in range(NC):
        perm = perms[c]
        dg = pdeg[c]

        # ---- segment tables + renumber maps ----
        tbls = np.zeros((Q, TBL, D), dtype=BF)
        remap = {}
        for q in range(Q):
            u = uniqs[(c, q)]
            tbls[q, 1:1 + len(u)] = h_bf_full[u]
            remap[q] = u            # sorted; renum = searchsorted+1

        def renum(q, srcs):
            return np.searchsorted(remap[q], srcs) + 1

        # ---- mailbox staging (wrapped int16, slot-major) + eig/mask ----
        idx_mail_w = np.zeros((128, SM_tot * 8), dtype=np.int16)
        a_mail = np.zeros((128, SM_tot), dtype=BF)
        mask_mail = np.zeros((128, SM_tot), dtype=BF)
        for b in range(NBLK):
            S_b, off = S_bs[b], moff[b]
            q = seg_of_block[b]
            flat = np.zeros((S_b, BLK), dtype=np.int64)   # [slot, dst]
            for d in range(BLK):
                r = b * BLK + d
                g = perm[r]
                k = dg[r]
                if g < 0 or k == 0:
                    continue
                raw = esrc_s[row_start[g]:row_start[g] + k]
                srcs = renum(q, raw)
                flat[:k, d] = srcs
                flat[k:, d] = srcs[0]
                a_mail[d, off:off + k] = eig0_bf[raw]
                mask_mail[d, off:off + k] = 1.0
            idx_mail_w[:, off * 8:(off + S_b) * 8] = _wrap16(flat.ravel())

        # ---- per-node scalars / residual ----
        safe = np.clip(perm, 0, N - 1)
        degf = dg.astype(np.float32)
        deg_t = np.ascontiguousarray(degf.reshape(NBLK, BLK).T)
        sn = np.where(perm >= 0, snorm_n[safe, 0], 0.0).astype(np.float32)
        snorm_t = np.ascontiguousarray(sn.reshape(NBLK, BLK).T)
        e0 = np.where(perm >= 0, eig0_bf[safe].astype(np.float32), 0.0)
        eig0d_t = np.ascontiguousarray(e0.reshape(NBLK, BLK).T).astype(np.float32)
        hin = np.where(perm[:, None] >= 0, h[safe], 0.0).astype(np.float32)

        m = dict(
            idx_mail=idx_mail_w, a_mail=a_mail, mask_mail=mask_mail,
            deg_t=deg_t, snorm_t=snorm_t, eig0d_t=eig0d_t, hin=hin,
        )
        for q in range(Q):
            m[f"tbl{q}"] = tbls[q]
        in_maps.append(m)

    meta = dict(perms=perms, S_bs=S_bs, moff=moff, SM_tot=SM_tot, Q=Q,
                seg_of_block=seg_of_block)
    return in_maps, meta


def _stage_consts(W, b, bn_gamma, bn_beta, bn_mean, bn_var):
    # W rows: c = i*384 + j*128 + f' (i = scale 0:id,1:amp,2:att;
    # j = agg 0:mean,1:max,2:dir).  wcat[:, j, i*128+f] = W[i*384+j*128+c, f]
    Wr = W.reshape(3, 3, 128, D)            # [i, j, c, f]
    wcat = np.ascontiguousarray(Wr.transpose(2, 1, 0, 3)).reshape(128, 3, 3 * D)
    bn = np.concatenate([bn_gamma, bn_beta, bn_mean, bn_var]).reshape(1, 4 * D)
    return dict(
        wcat=wcat.astype(np.float32),
        bvec=b.reshape(1, D).astype(np.float32),
        bn=bn.astype(np.float32),
        ident_bf=np.eye(128, dtype=BF),
    )


def _build_program(cfg, meta):
    NBLK, NPC_PAD = cfg.NBLK, cfg.NPC_PAD
    S_bs, moff = meta["S_bs"], meta["moff"]
    SM_tot, Q = meta["SM_tot"], meta["Q"]
    seg_of_block = meta["seg_of_block"]
    Smax = max(S_bs)
    AOT = mybir.AluOpType
    AFT = mybir.ActivationFunctionType

    nc = bacc.Bacc("TRN2", target_bir_lowering=False, debug=False)

    tbl_d = [nc.dram_tensor(f"tbl{q}", [TBL, D], BF16, kind="ExternalInput")
             for q in range(Q)]
    idx_mail = nc.dram_tensor("idx_mail", [128, SM_tot * 8], I16,
                              kind="ExternalInput")
    a_mail_d = nc.dram_tensor("a_mail", [128, SM_tot], BF16, kind="ExternalInput")
    mask_d = nc.dram_tensor("mask_mail", [128, SM_tot], BF16, kind="ExternalInput")
    deg_t = nc.dram_tensor("deg_t", [128, NBLK], F32, kind="ExternalInput")
    snorm_t = nc.dram_tensor("snorm_t", [128, NBLK], F32, kind="ExternalInput")
    eig0d_t = nc.dram_tensor("eig0d_t", [128, NBLK], F32, kind="ExternalInput")
    hin = nc.dram_tensor("hin", [NPC_PAD, D], F32, kind="ExternalInput")
    wcat = nc.dram_tensor("wcat", [128, 3, 3 * D], F32, kind="ExternalInput")
    bvec = nc.dram_tensor("bvec", [1, D], F32, kind="ExternalInput")
    bn = nc.dram_tensor("bn", [1, 4 * D], F32, kind="ExternalInput")
    ident_bf_d = nc.dram_tensor("ident_bf", [128, 128], BF16, kind="ExternalInput")

    out_d = nc.dram_tensor("out", [NPC_PAD, D], F32, kind="ExternalOutput")

    with tile.TileContext(nc) as tc:
        with (
            tc.tile_pool(name="stage", bufs=1) as stg,
            tc.tile_pool(name="const", bufs=1) as cst,
            tc.tile_pool(name="idxp", bufs=3) as idxp,
            tc.tile_pool(name="mail", bufs=3) as mailp,
            tc.tile_pool(name="diag", bufs=2) as diagp,
            tc.tile_pool(name="work", bufs=3) as wk,
            tc.tile_pool(name="ep", bufs=2) as ep,
            tc.tile_pool(name="pagg", bufs=2, space="PSUM") as pagg,
            tc.tile_pool(name="ptp", bufs=2, space="PSUM") as ptp,
            tc.tile_pool(name="py", bufs=2, space="PSUM") as py,
        ):
            nc.gpsimd.load_library(mlp)

            # ---------- staging loads ----------
            def load(dram, shape, dtype, pool=stg):
                t = pool.tile(shape, dtype, tag=dram.name)
                nc.sync.dma_start(t[:], dram[:])
                return t

            amail_s = load(a_mail_d, [128, SM_tot], BF16)
            mask_s = load(mask_d, [128, SM_tot], BF16)
            degt_s = load(deg_t, [128, NBLK], F32)
            snormt_s = load(snorm_t, [128, NBLK], F32)
            eig0dt_s = load(eig0d_t, [128, NBLK], F32)
            bvec_s = load(bvec, [1, D], F32)
            bn_s = load(bn, [1, 4 * D], F32)
            identbf_s = load(ident_bf_d, [128, 128], BF16, pool=cst)
            wcat_s = load(wcat, [128, 3, 3 * D], F32)

            # ---------- bn fold / constant prep (rows on partition 0) ----------
            g_r = bn_s[:, 0:D]
            beta_r = bn_s[:, D:2 * D]
            mean_r = bn_s[:, 2 * D:3 * D]
            var_r = bn_s[:, 3 * D:4 * D]
            bnsc = cst.tile([1, D], F32, tag="bnsc")
            eps_t = cst.tile([1, 1], F32, tag="eps_t")
            nc.gpsimd.memset(eps_t[:], BN_EPS)
            nc.scalar.activation(bnsc[:], var_r, AFT.Sqrt, bias=eps_t[:], scale=1.0)
            nc.vector.reciprocal(bnsc[:], bnsc[:])
            nc.vector.tensor_tensor(bnsc[:], bnsc[:], g_r, op=AOT.mult)
            shift = cst.tile([1, D], F32, tag="shift")       # beta - mean*scale
            nc.vector.tensor_tensor(shift[:], mean_r, bnsc[:], op=AOT.mult)
            nc.vector.tensor_tensor(shift[:], beta_r, shift[:], op=AOT.subtract)
            bprime = cst.tile([1, D], F32, tag="bprime")     # b * scale
            nc.vector.tensor_tensor(bprime[:], bvec_s[:], bnsc[:], op=AOT.mult)

            # broadcast const rows across partitions (DMA replicate via DRAM)
            rows_dram = nc.dram_tensor("cst_rows", [3, D], F32)
            nc.sync.dma_start(rows_dram[0:1, :], bnsc[:])
            nc.sync.dma_start(rows_dram[1:2, :], shift[:])
            nc.sync.dma_start(rows_dram[2:3, :], bprime[:])
            bnsc_bc = cst.tile([128, D], F32, tag="bnsc_bc")
            nc.sync.dma_start(bnsc_bc[:], rows_dram[0:1, :].to_broadcast([128, D]))
            shift_bc = cst.tile([128, D], F32, tag="shift_bc")
            nc.sync.dma_start(shift_bc[:], rows_dram[1:2, :].to_broadcast([128, D]))
            bprime_bc = cst.tile([128, D], F32, tag="bprime_bc")
            nc.sync.dma_start(bprime_bc[:], rows_dram[2:3, :].to_broadcast([128, D]))

            # wcat_bf = wcat * bn_scale -> bf16
            wcat_bf = cst.tile([128, 3, 3 * D], BF16, tag="wcatbf")
            nc.vector.tensor_tensor(
                wcat_bf[:].rearrange("p j (i d) -> p j i d", i=3),
                wcat_s[:].rearrange("p j (i d) -> p j i d", i=3),
                bnsc_bc[:, None, None, :].to_broadcast([128, 3, 3, D]),
                op=AOT.mult)

            for b in range(NBLK):
                S_b, mo = S_bs[b], int(moff[b])
                tdram = tbl_d[int(seg_of_block[b])]

                # ======== mailbox gather (issued first; prep below is
                # independent of it and overlaps) ========
                im = idxp.tile([128, Smax * 8], I16, tag="im")
                nc.sync.dma_start(im[:, 0:S_b * 8],
                                  idx_mail[:, mo * 8:(mo + S_b) * 8])
                mail = mailp.tile([128, Smax, D], BF16, tag="mail")
                nc.gpsimd.dma_gather(mail[:, 0:S_b, :], tdram[:],
                                     im[:, 0:S_b * 8], S_b * 128, S_b * 128,
                                     D, single_packet=False)

                # ======== edge weights w = mask * |a_src - a_dst| ========
                wv = wk.tile([128, Smax], BF16, tag="wv")
                nc.vector.tensor_scalar(wv[:, 0:S_b], amail_s[:, mo:mo + S_b],
                                        eig0dt_s[:, b:b + 1], None,
                                        op0=AOT.subtract)
                nc.scalar.activation(wv[:, 0:S_b], wv[:, 0:S_b], AFT.Abs)
                nc.vector.tensor_tensor(wv[:, 0:S_b], wv[:, 0:S_b],
                                        mask_s[:, mo:mo + S_b], op=AOT.mult)
                den = wk.tile([128, 1], F32, tag="den")
                nc.vector.tensor_reduce(den[:], wv[:, 0:S_b],
                                        axis=mybir.AxisListType.X, op=AOT.add)
                nc.vector.tensor_scalar(den[:], den[:], 1e-30, None, op0=AOT.add)
                rden = wk.tile([128, 1], F32, tag="rden")
                nc.vector.reciprocal(rden[:], den[:])

                # diag_all[:, s, :] = diag weights for slot s (identity * w_s)
                diag = diagp.tile([128, Smax, 128], BF16, tag="diag")
                nc.vector.tensor_tensor(
                    diag[:, 0:S_b, :],
                    identbf_s[:, None, :].to_broadcast([128, S_b, 128]),
                    wv[:, 0:S_b, None].to_broadcast([128, S_b, 128]),
                    op=AOT.mult)

                # ======== PE slot accumulation: sum and dir ========
                ps_sum = pagg.tile([128, D], F32, tag="ps_sum")
                ps_dir = pagg.tile([128, D], F32, tag="ps_dir")
                for s in range(S_b):
                    nc.tensor.matmul(ps_sum[:], identbf_s[:], mail[:, s, :],
                                     start=(s == 0), stop=(s == S_b - 1),
                                     skip_group_check=True)
                for s in range(S_b):
                    nc.tensor.matmul(ps_dir[:], diag[:, s, :], mail[:, s, :],
                                     start=(s == 0), stop=(s == S_b - 1),
                                     skip_group_check=True)

                # ======== per-node scalar columns ========
                deg_col = degt_s[:, b:b + 1]
                degsafe = wk.tile([128, 1], F32, tag="degsafe")
                nc.vector.tensor_scalar(degsafe[:], deg_col, 1.0, None,
                                        op0=AOT.max)
                nrn = wk.tile([128, 1], F32, tag="nrn")      # -1/deg_safe
                nc.vector.reciprocal(nrn[:], degsafe[:])
                nc.vector.tensor_scalar(nrn[:], nrn[:], -1.0, None, op0=AOT.mult)
                cnt_col = wk.tile([128, 1], F32, tag="cnt")  # S_b - deg
                nc.vector.tensor_scalar(cnt_col[:], deg_col, -1.0, float(S_b),
                                        op0=AOT.mult, op1=AOT.add)
                logd_col = wk.tile([128, 1], F32, tag="logd_col")
                nc.scalar.activation(logd_col[:], deg_col, AFT.Ln,
                                     bias=1.0, scale=1.0)
                amp_col = wk.tile([128, 1], F32, tag="amp_col")
                nc.vector.tensor_scalar(amp_col[:], logd_col[:],
                                        1.0 / AVG_D_LOG, None, op0=AOT.mult)
                att_col = wk.tile([128, 1], F32, tag="att_col")
                nc.vector.tensor_scalar(att_col[:], logd_col[:], 1e-6, None,
                                        op0=AOT.max)
                nc.vector.reciprocal(att_col[:], att_col[:])
                nc.vector.tensor_scalar(att_col[:], att_col[:], AVG_D_LOG, None,
                                        op0=AOT.mult)

                # ======== padding correction + scaling -> bf16 [d, f] ========
                # mean = -(cnt*mail0 - sum) / deg_safe
                tmp = ep.tile([128, D], F32, tag="tmp")
                nc.vector.scalar_tensor_tensor(
                    tmp[:], mail[:, 0, :], cnt_col[:], ps_sum[:],
                    op0=AOT.mult, op1=AOT.subtract)
                mean_bf = ep.tile([128, D], BF16, tag="mean_bf")
                nc.vector.tensor_scalar(mean_bf[:], tmp[:], nrn[:], None,
                                        op0=AOT.mult)
                dir_bf = ep.tile([128, D], BF16, tag="dir_bf")
                nc.vector.tensor_scalar(dir_bf[:], ps_dir[:], rden[:], None,
                                        op0=AOT.mult)

                # ======== max tree (in place; after mail0 was consumed) ========
                n = S_b
                while n > 1:
                    h1 = (n + 1) // 2
                    nc.vector.tensor_tensor(
                        mail[:, 0:h1, :], mail[:, 0:h1, :],
                        mail[:, n - h1:n, :], op=AOT.max)
                    n = h1

                # ======== transpose [d,f] -> [f,d] ========
                rawT = []
                for j, src in enumerate((mean_bf[:], mail[:, 0, :], dir_bf[:])):
                    tp = ptp.tile([128, 128], BF16, tag="tp")
                    nc.tensor.transpose(tp[:], src, identbf_s[:])
                    rt = ep.tile([128, 128], BF16, tag=f"rawT{j}")
                    nc.scalar.copy(rt[:], tp[:])
                    rawT.append(rt)

                # ======== final matmuls + combine ========
                y_ps = py.tile([128, 3 * D], F32, tag="y")
                for j, rt in enumerate(rawT):
                    nc.tensor.matmul(y_ps[:], rt[:], wcat_bf[:, j, :],
                                     start=(j == 0), stop=(j == 2))

                y1_sb = ep.tile([128, D], F32, tag="y1_sb")
                nc.scalar.copy(y1_sb[:], y_ps[:, 0:D])
                u = ep.tile([128, D], F32, tag="u")
                nc.vector.scalar_tensor_tensor(
                    u[:], y_ps[:, D:2 * D], amp_col[:], y1_sb[:],
                    op0=AOT.mult, op1=AOT.add)
                v = ep.tile([128, D], F32, tag="v")
                nc.vector.scalar_tensor_tensor(
                    v[:], y_ps[:, 2 * D:3 * D], att_col[:], u[:],
                    op0=AOT.mult, op1=AOT.add)
                nc.vector.tensor_tensor(v[:], v[:], bprime_bc[:], op=AOT.add)
                nc.vector.scalar_tensor_tensor(
                    v[:], v[:], snormt_s[:, b:b + 1], shift_bc[:],
                    op0=AOT.mult, op1=AOT.add)
                hin_t = ep.tile([128, D], F32, tag="hin")
                nc.sync.dma_start(hin_t[:], hin[b * BLK:(b + 1) * BLK, :])
                out_t = ep.tile([128, D], F32, tag="out")
                nc.vector.scalar_tensor_tensor(
                    out_t[:], v[:], 0.0, hin_t[:], op0=AOT.max, op1=AOT.add)
                nc.sync.dma_start(out_d[b * BLK:(b + 1) * BLK, :], out_t[:])

    nc.compile()
    return nc


_CACHE = {}


def _run(h, eig, snorm_n, W, b, bn_gamma, bn_beta, bn_mean, bn_var,
         edge_src, edge_dst, n_cores=8, trace=False, sim=False):
    N, E = h.shape[0], edge_src.shape[0]
    cfg = _Cfg(N, E, n_cores)
    in_maps, meta = _preprocess(cfg, h, eig, snorm_n, edge_src, edge_dst)
    consts = _stage_consts(W, b, bn_gamma, bn_beta, bn_mean, bn_var)
    for m in in_maps:
        m.update(consts)

    key = (N, E, n_cores, meta["Q"], tuple(meta["S_bs"]))
    if key not in _CACHE:
        _CACHE[key] = _build_program(cfg, meta)
    nc = _CACHE[key]

    if sim:
        from concourse.bass_interp import CoreSim
        csim = CoreSim(nc)
        for k, v in in_maps[0].items():
            csim.tensor(k)[:] = v
        csim.simulate()
        results = [{"out": np.array(csim.tensor("out"))}]
        n_out = 1
        res = None
    else:
        res = run_bass_kernel_spmd(nc, in_maps, core_ids=list(range(n_cores)),
                                   trace=trace)
        results = res.results
        n_out = n_cores

    out = np.empty((N, D), dtype=np.float32)
    for c in range(n_out):
        perm = meta["perms"][c]
        oc = results[c]["out"]
        valid = perm >= 0
        out[perm[valid]] = oc[valid]
    return out, res


def kernel(**inputs):
    out, _ = _run(
        np.asarray(inputs["h"]), np.asarray(inputs["eig"]),
        np.asarray(inputs["snorm_n"]), np.asarray(inputs["W"]),
        np.asarray(inputs["b"]), np.asarray(inputs["bn_gamma"]),
        np.asarray(inputs["bn_beta"]), np.asarray(inputs["bn_mean"]),
        np.asarray(inputs["bn_var"]), np.asarray(inputs["edge_src"]),
        np.asarray(inputs["edge_dst"]))
    return out
